# revision 1
# baseline (speedup 1.0000x reference)
"""HashEmbedder (HashNeRF multires hash encoding) Trainium2 kernel.

Strategy:
 - Only levels 0..7 survive the reference's crop to 16 output columns
   (16 levels x 2 feats = 32 -> [:, :16]), so levels 8..15 are skipped.
 - Level-sharded across the 8 NeuronCores: core l handles level l for all
   1M points.
 - Per level, the hash table is re-laid-out host-side into a dense VOXEL
   table V[(R+1)^3, 16] whose 64B rows hold all 8 corner embeddings of one
   voxel (i-major corner order, feats innermost). This is a weight-layout
   transform (like pre-transposing matmul weights): the device kernel then
   needs exactly one 64B gather per point and no hashing at all.
 - Device kernel: floor/frac in f32, voxel index arithmetic in f32 (exact:
   values < 2^24), one indirect-DMA gather per point, trilinear lerp
   cascade, write [N, 2] per core; host concatenates the 8 cores' columns.
"""
import sys
import numpy as np

sys.path.insert(0, "/opt/trn_rl_repo")

import concourse.bass as bass
import concourse.tile as tile
from concourse import bacc, mybir
from concourse.bass_utils import run_bass_kernel_spmd
from contextlib import ExitStack

# ---- problem constants (hardcoded; kernel.py must be self-contained) ----
N_POINTS = 1048576
LOG2_T = 19
TABLE_SIZE = 1 << LOG2_T
NFPL = 2
BASE_RES = 16.0
FINEST_RES = 512.0
N_LEVELS_TOTAL = 16
N_LEVELS_USED = 8

_b = np.exp((np.log(FINEST_RES) - np.log(BASE_RES)) / (N_LEVELS_TOTAL - 1))
RES = [int(np.floor(np.float32(BASE_RES) * np.float32(_b) ** np.float32(l)))
       for l in range(N_LEVELS_USED)]  # [16, 20, 25, 32, 40, 50, 64, 80]
VD = [r + 1 for r in RES]              # voxel grid dim per axis (bl in [0, R])
VMAX = max(d ** 3 for d in VD)         # padded voxel-table rows (81^3)
VMAX2 = (VMAX + 1) // 2                # voxel-pair rows (128B each)

P = 128
PPP = N_POINTS // P   # points per partition (8192)
CHUNK = 256           # points per partition per iteration

_PRIMES = np.array([1, 2654435761, 805459861], dtype=np.uint64)

_COMPILED = None


def _build_voxel_tables(tables: np.ndarray) -> list:
    """V[l][vox, 16]: vox = (vz*(R+1) + vy)*(R+1) + vx, row layout
    [i][j][k][f] (x-offset-major corners, feats innermost)."""
    out = []
    for l in range(N_LEVELS_USED):
        D = VD[l]
        tab = tables[l]  # [TABLE_SIZE, 2] float32
        # vertex hash grid: verts 0..D (need bl+1 <= D)
        vs = np.arange(D + 1, dtype=np.uint64)
        hx = (vs * _PRIMES[0])[:, None, None]
        hy = (vs * _PRIMES[1])[None, :, None]
        hz = (vs * _PRIMES[2])[None, None, :]
        h = (hx ^ hy ^ hz) & np.uint64(TABLE_SIZE - 1)   # [D+1, D+1, D+1]
        dense = tab[h.astype(np.int64)]                   # [D+1, D+1, D+1, 2]
        V = np.empty((D, D, D, 8, 2), dtype=np.float32)
        for ci, i in enumerate((0, 1)):
            for cj, j in enumerate((0, 1)):
                for ck, k in enumerate((0, 1)):
                    c = 4 * ci + 2 * cj + ck
                    # vox index (vz,vy,vx) nesting -> dense[x+i, y+j, z+k]
                    V[:, :, :, c, :] = np.transpose(
                        dense[i:i + D, j:j + D, k:k + D], (2, 1, 0, 3))
        V = V.reshape(D * D * D, 16)
        if V.shape[0] < 2 * VMAX2:
            V = np.concatenate(
                [V, np.zeros((2 * VMAX2 - V.shape[0], 16), np.float32)],
                axis=0)
        out.append(np.ascontiguousarray(V.reshape(VMAX2, 32)))
    return out


def _compile():
    nc = bacc.Bacc("TRN2", target_bir_lowering=False, debug=False,
                   num_devices=8)
    x_d = nc.dram_tensor("x", [N_POINTS, 3], mybir.dt.float32,
                         kind="ExternalInput").ap()
    v_d = nc.dram_tensor("vtab", [VMAX2, 32], mybir.dt.float32,
                         kind="ExternalInput").ap()
    c_d = nc.dram_tensor("consts", [P, 1, 4], mybir.dt.float32,
                         kind="ExternalInput").ap()
    o_d = nc.dram_tensor("out", [N_POINTS, 2], mybir.dt.float32,
                         kind="ExternalOutput").ap()

    xr = x_d.rearrange("(p n) d -> p n d", p=P)   # [128, PPP, 3]
    orr = o_d.rearrange("(p n) d -> p n d", p=P)  # [128, PPP, 2]

    f32 = mybir.dt.float32
    i32 = mybir.dt.int32
    A = mybir.AluOpType

    with tile.TileContext(nc) as tc:
        with ExitStack() as ctx:
            cpool = ctx.enter_context(tc.tile_pool(name="consts", bufs=1))
            xpool = ctx.enter_context(tc.tile_pool(name="x", bufs=3))
            gpool = ctx.enter_context(tc.tile_pool(name="g", bufs=2))
            wpool = ctx.enter_context(tc.tile_pool(name="w", bufs=2))

            ct = cpool.tile([P, 1, 4], f32)
            nc.sync.dma_start(out=ct[:], in_=c_d[:])
            rt = ct[:, :, 0:1]    # R
            c3 = ct[:, :, 1:4]    # [1, R+1, (R+1)^2]

            for it in range(PPP // CHUNK):
                m = CHUNK
                xt = xpool.tile([P, m, 3], f32)
                nc.sync.dma_start(out=xt[:], in_=xr[:, it * m:(it + 1) * m, :])

                t = wpool.tile([P, m, 3], f32, tag="t")
                nc.vector.tensor_tensor(out=t[:], in0=xt[:],
                                        in1=rt.to_broadcast([P, m, 3]),
                                        op=A.mult)
                ti = wpool.tile([P, m, 3], i32, tag="ti")
                nc.scalar.copy(out=ti[:], in_=t[:])       # round-to-nearest
                bf = wpool.tile([P, m, 3], f32, tag="bf")
                nc.scalar.copy(out=bf[:], in_=ti[:])
                fx = wpool.tile([P, m, 3], f32, tag="fx")
                nc.vector.tensor_tensor(out=fx[:], in0=bf[:], in1=t[:],
                                        op=A.is_gt)      # 1.0 where rounded up
                nc.vector.tensor_tensor(out=bf[:], in0=bf[:], in1=fx[:],
                                        op=A.subtract)   # bf = exact floor(t)
                nc.vector.tensor_tensor(out=t[:], in0=t[:], in1=bf[:],
                                        op=A.subtract)   # t = frac weights w
                nc.vector.tensor_tensor(out=fx[:], in0=bf[:],
                                        in1=c3.to_broadcast([P, m, 3]),
                                        op=A.mult)       # fx = bf * [1,R1,R1^2]
                voxf = wpool.tile([P, m, 1], f32, tag="voxf")
                nc.vector.tensor_reduce(out=voxf[:], in_=fx[:],
                                        axis=mybir.AxisListType.X, op=A.add)
                # pair row w = floor(vox/2), parity sel = vox - 2w (exact f32)
                hf = wpool.tile([P, m, 1], f32, tag="hf")
                nc.vector.tensor_scalar_mul(out=hf[:], in0=voxf[:],
                                            scalar1=0.5)
                hi = wpool.tile([P, m, 1], i32, tag="hi")
                nc.scalar.copy(out=hi[:], in_=hf[:])      # rne(vox/2)
                hc = wpool.tile([P, m, 1], f32, tag="hc")
                nc.scalar.copy(out=hc[:], in_=hi[:])
                hx = wpool.tile([P, m, 1], f32, tag="hx")
                nc.vector.tensor_tensor(out=hx[:], in0=hc[:], in1=hf[:],
                                        op=A.is_gt)
                nc.vector.tensor_tensor(out=hc[:], in0=hc[:], in1=hx[:],
                                        op=A.subtract)    # hc = floor(vox/2)
                sel = wpool.tile([P, m, 1], f32, tag="sel")
                nc.vector.tensor_scalar_mul(out=sel[:], in0=hc[:],
                                            scalar1=-2.0)
                nc.vector.tensor_tensor(out=sel[:], in0=voxf[:], in1=sel[:],
                                        op=A.add)         # sel = vox - 2w
                voxi = wpool.tile([P, m, 1], i32, tag="voxi")
                nc.scalar.copy(out=voxi[:], in_=hc[:])    # pair row index

                g = gpool.tile([P, m, 32], f32, tag="g")
                for j in range(m):
                    nc.gpsimd.indirect_dma_start(
                        out=g[:, j, :],
                        out_offset=None,
                        in_=v_d[:],
                        in_offset=bass.IndirectOffsetOnAxis(
                            ap=voxi[:, j, :], axis=0),
                    )

                # parity select: g[0:16] = g[0:16] + (g[16:32]-g[0:16])*sel
                nc.vector.tensor_tensor(out=g[:, :, 16:32], in0=g[:, :, 16:32],
                                        in1=g[:, :, 0:16], op=A.subtract)
                nc.vector.tensor_tensor(out=g[:, :, 16:32], in0=g[:, :, 16:32],
                                        in1=sel.to_broadcast([P, m, 16]),
                                        op=A.mult)
                nc.vector.tensor_tensor(out=g[:, :, 0:16], in0=g[:, :, 0:16],
                                        in1=g[:, :, 16:32], op=A.add)

                # trilinear cascade in place: x, then y, then z; result g[...,0:2]
                nc.vector.tensor_tensor(out=g[:, :, 8:16], in0=g[:, :, 8:16],
                                        in1=g[:, :, 0:8], op=A.subtract)
                nc.vector.tensor_tensor(out=g[:, :, 8:16], in0=g[:, :, 8:16],
                                        in1=t[:, :, 0:1].to_broadcast([P, m, 8]),
                                        op=A.mult)
                nc.vector.tensor_tensor(out=g[:, :, 0:8], in0=g[:, :, 0:8],
                                        in1=g[:, :, 8:16], op=A.add)

                nc.vector.tensor_tensor(out=g[:, :, 4:8], in0=g[:, :, 4:8],
                                        in1=g[:, :, 0:4], op=A.subtract)
                nc.vector.tensor_tensor(out=g[:, :, 4:8], in0=g[:, :, 4:8],
                                        in1=t[:, :, 1:2].to_broadcast([P, m, 4]),
                                        op=A.mult)
                nc.vector.tensor_tensor(out=g[:, :, 0:4], in0=g[:, :, 0:4],
                                        in1=g[:, :, 4:8], op=A.add)

                nc.vector.tensor_tensor(out=g[:, :, 2:4], in0=g[:, :, 2:4],
                                        in1=g[:, :, 0:2], op=A.subtract)
                nc.vector.tensor_tensor(out=g[:, :, 2:4], in0=g[:, :, 2:4],
                                        in1=t[:, :, 2:3].to_broadcast([P, m, 2]),
                                        op=A.mult)
                nc.vector.tensor_tensor(out=g[:, :, 0:2], in0=g[:, :, 0:2],
                                        in1=g[:, :, 2:4], op=A.add)

                nc.sync.dma_start(out=orr[:, it * m:(it + 1) * m, :],
                                  in_=g[:, :, 0:2])

    nc.compile()
    return nc


def _get_compiled():
    global _COMPILED
    if _COMPILED is None:
        _COMPILED = _compile()
    return _COMPILED


def kernel(x: np.ndarray, tables: np.ndarray, _want_trace: bool = False):
    nc = _get_compiled()
    x = np.ascontiguousarray(np.asarray(x, dtype=np.float32))
    tables = np.asarray(tables, dtype=np.float32)
    vs = _build_voxel_tables(tables)
    in_maps = []
    for l in range(N_LEVELS_USED):
        r1 = float(RES[l] + 1)
        consts = np.tile(
            np.array([[[float(RES[l]), 1.0, r1, r1 * r1]]], np.float32), (P, 1, 1))
        in_maps.append({"x": x, "vtab": vs[l], "consts": consts})
    res = run_bass_kernel_spmd(nc, in_maps, list(range(8)),
                               trace=_want_trace)
    out = np.empty((N_POINTS, 16), dtype=np.float32)
    for l in range(N_LEVELS_USED):
        # device wrote [128, PPP, 2] flattened as [N, 2] in (p, n) order
        out[:, 2 * l:2 * l + 2] = res.results[l]["out"]
    if _want_trace:
        return out, res
    return out



# revision 6
# speedup vs baseline: 3.1338x; 3.1338x over previous
"""HashEmbedder (HashNeRF multires hash encoding) Trainium2 kernel.

The graded metric is the warm wall-clock of a full kernel() call, which under
the axon tunnel (~20-50 MB/s h2d+d2h) is dominated by bytes moved, not device
compute. Strategy:

 - Only levels 0..7 survive the reference's crop to 16 output columns.
 - POINT-sharded across the 8 NeuronCores: core c handles points
   [c*131072, (c+1)*131072) for all 8 levels, so x is shipped once (12 MB
   total) instead of replicated 8x.
 - Per level, the hash table is collapsed host-side into a DENSE VERTEX grid
   D_l[(R+1)^3, 2] = table_l[hash(vx,vy,vz)] (input-independent hash grids are
   precomputed once).  All 8 levels pack into one [TOTAL_VERTS, 2] bf16 buffer
   (~4.3 MB per core).  Layout [vx][vy][vz][feat] makes the two z-corners of a
   voxel contiguous (8 bytes), so each of the 4 (x,y) corner pairs is one
   gather row - no corner duplication, no hashing on device.
 - Outputs are written bf16 (32 MB total instead of 64) and upcast on host.
 - Device kernel: floor/frac in f32 (exact: values < 2^24), vertex index
   arithmetic in f32, 4 multi-offset indirect-DMA gathers per (chunk, level),
   trilinear lerp cascade in f32, pack to bf16.
"""
import os
import sys
import numpy as np

sys.path.insert(0, "/opt/trn_rl_repo")

import concourse.bass as bass
import concourse.tile as tile
from concourse import bacc, mybir
from concourse.bass_utils import run_bass_kernel_spmd
from contextlib import ExitStack

import ml_dtypes

BF16 = ml_dtypes.bfloat16

# ---- problem constants (hardcoded; kernel.py must be self-contained) ----
N_POINTS = 1048576
LOG2_T = 19
TABLE_SIZE = 1 << LOG2_T
BASE_RES = 16.0
FINEST_RES = 512.0
N_LEVELS_TOTAL = 16
N_LEVELS_USED = 8
N_CORES = 8

_b = np.exp((np.log(FINEST_RES) - np.log(BASE_RES)) / (N_LEVELS_TOTAL - 1))
RES = [int(np.floor(np.float32(BASE_RES) * np.float32(_b) ** np.float32(l)))
       for l in range(N_LEVELS_USED)]  # [16, 20, 25, 32, 40, 50, 64, 80]
VD = [r + 1 for r in RES]              # vertex grid dim per axis
NVERT = [d ** 3 for d in VD]
VBASE = [int(x) for x in np.cumsum([0] + NVERT)[:N_LEVELS_USED]]
TOTAL_VERTS = sum(NVERT)               # 1,075,325

P = 128
NPC = int(os.environ.get("BASSK_NPC", N_POINTS // N_CORES))  # points per core
PPP = NPC // P                         # points per partition per core
CHUNK = min(256, PPP)                  # points per partition per iteration
JB = int(os.environ.get("BASSK_JB", 32))  # gather offsets per indirect-DMA instr

_PRIMES = np.array([1, 2654435761, 805459861], dtype=np.uint64)

_COMPILED = None
_HGRIDS = None


def _hash_grids():
    """Input-independent per-level hash index grids H_l[(R+1)^3] int32."""
    global _HGRIDS
    if _HGRIDS is None:
        gs = []
        for l in range(N_LEVELS_USED):
            D = VD[l]
            vs = np.arange(D, dtype=np.uint64)
            hx = (vs * _PRIMES[0])[:, None, None]
            hy = (vs * _PRIMES[1])[None, :, None]
            hz = (vs * _PRIMES[2])[None, None, :]
            h = (hx ^ hy ^ hz) & np.uint64(TABLE_SIZE - 1)
            gs.append(h.astype(np.int32).ravel())
        _HGRIDS = gs
    return _HGRIDS


def _build_packed_table(tables: np.ndarray) -> np.ndarray:
    """[TOTAL_VERTS, 2] bf16: all levels' dense vertex grids, x-major."""
    grids = _hash_grids()
    packed = np.empty((TOTAL_VERTS, 2), dtype=np.float32)
    for l in range(N_LEVELS_USED):
        packed[VBASE[l]:VBASE[l] + NVERT[l]] = tables[l][grids[l]]
    return packed.astype(BF16)


def _compile():
    nc = bacc.Bacc("TRN2", target_bir_lowering=False, debug=False,
                   num_devices=N_CORES)
    x_d = nc.dram_tensor("x", [NPC, 3], mybir.dt.float32,
                         kind="ExternalInput").ap()
    v_d = nc.dram_tensor("vtab", [TOTAL_VERTS, 2], mybir.dt.bfloat16,
                         kind="ExternalInput").ap()
    c_d = nc.dram_tensor("consts", [P, 1, 3 * N_LEVELS_USED], mybir.dt.float32,
                         kind="ExternalInput").ap()
    o_d = nc.dram_tensor("out", [NPC, 16], mybir.dt.bfloat16,
                         kind="ExternalOutput").ap()

    xr = x_d.rearrange("(p n) d -> p n d", p=P)   # [128, PPP, 3]
    orr = o_d.rearrange("(p n) d -> p n d", p=P)  # [128, PPP, 16]

    f32 = mybir.dt.float32
    i32 = mybir.dt.int32
    bf16 = mybir.dt.bfloat16
    A = mybir.AluOpType

    with tile.TileContext(nc) as tc:
        with ExitStack() as ctx:
            cpool = ctx.enter_context(tc.tile_pool(name="consts", bufs=1))
            xpool = ctx.enter_context(tc.tile_pool(name="x", bufs=2))
            opool = ctx.enter_context(tc.tile_pool(name="o", bufs=2))
            gpool = ctx.enter_context(tc.tile_pool(name="g", bufs=2))
            wpool = ctx.enter_context(tc.tile_pool(name="w", bufs=2))

            ct = cpool.tile([P, 1, 3 * N_LEVELS_USED], f32)
            nc.sync.dma_start(out=ct[:], in_=c_d[:])

            m = CHUNK
            for it in range(PPP // m):
                xt = xpool.tile([P, m, 3], f32, tag="xt")
                nc.sync.dma_start(out=xt[:], in_=xr[:, it * m:(it + 1) * m, :])
                ot = opool.tile([P, m, 16], bf16, tag="ot")

                for l in range(N_LEVELS_USED):
                    c3 = ct[:, :, 3 * l:3 * l + 3]   # [S1, S2, 1]
                    t = wpool.tile([P, m, 3], f32, tag="t")
                    nc.vector.tensor_scalar_mul(out=t[:], in0=xt[:],
                                                scalar1=float(RES[l]))
                    ti = wpool.tile([P, m, 3], i32, tag="ti")
                    nc.scalar.copy(out=ti[:], in_=t[:])    # round-to-nearest
                    bf = wpool.tile([P, m, 3], f32, tag="bf")
                    nc.scalar.copy(out=bf[:], in_=ti[:])
                    fx = wpool.tile([P, m, 3], f32, tag="fx")
                    nc.vector.tensor_tensor(out=fx[:], in0=bf[:], in1=t[:],
                                            op=A.is_gt)   # 1.0 where rounded up
                    nc.vector.tensor_tensor(out=bf[:], in0=bf[:], in1=fx[:],
                                            op=A.subtract)  # bf = floor(t)
                    nc.vector.tensor_tensor(out=t[:], in0=t[:], in1=bf[:],
                                            op=A.subtract)  # t = frac weights
                    nc.vector.tensor_tensor(out=fx[:], in0=bf[:],
                                            in1=c3.to_broadcast([P, m, 3]),
                                            op=A.mult)
                    vertf = wpool.tile([P, m, 1], f32, tag="vertf")
                    nc.vector.tensor_reduce(out=vertf[:], in_=fx[:],
                                            axis=mybir.AxisListType.X, op=A.add)

                    # 4 corner-pair index tiles: vert + base + i*S1 + j*S2
                    S1, S2 = VD[l] * VD[l], VD[l]
                    gp = gpool.tile([P, 4 * m, 4], bf16, tag="gp")
                    for pair in range(4):
                        i, j = pair >> 1, pair & 1
                        off = float(VBASE[l] + i * S1 + j * S2)
                        vp = wpool.tile([P, m, 1], f32, tag=f"vp{pair}")
                        nc.vector.tensor_scalar_add(out=vp[:], in0=vertf[:],
                                                    scalar1=off)
                        vi = wpool.tile([P, m, 1], i32, tag=f"vi{pair}")
                        nc.scalar.copy(out=vi[:], in_=vp[:])  # exact ints
                        jb = min(JB, m)
                        if jb == 1:
                            for j0 in range(m):
                                nc.gpsimd.indirect_dma_start(
                                    out=gp[:, pair * m + j0, :],
                                    out_offset=None,
                                    in_=v_d[:],
                                    in_offset=bass.IndirectOffsetOnAxis(
                                        ap=vi[:, j0, :], axis=0),
                                )
                        else:
                            for j0 in range(0, m, jb):
                                nc.gpsimd.indirect_dma_start(
                                    out=gp[:, pair * m + j0:pair * m + j0 + jb, :],
                                    out_offset=None,
                                    in_=v_d[:],
                                    in_offset=bass.IndirectOffsetOnAxis(
                                        ap=vi[:, j0:j0 + jb, 0:1], axis=0),
                                )

                    gf = gpool.tile([P, 4 * m, 4], f32, tag="gf")
                    nc.vector.tensor_copy(out=gf[:], in_=gp[:])

                    # z-lerp within each pair: zt[p] = g0 + wz*(g1 - g0)
                    zd = gpool.tile([P, 4 * m, 2], f32, tag="zd")
                    nc.vector.tensor_tensor(out=zd[:], in0=gf[:, :, 2:4],
                                            in1=gf[:, :, 0:2], op=A.subtract)
                    zt = gpool.tile([P, 4 * m, 2], f32, tag="zt")
                    wz = t[:, :, 2:3]
                    for pr in range(4):
                        s = slice(pr * m, (pr + 1) * m)
                        nc.vector.tensor_tensor(
                            out=zd[:, s, :], in0=zd[:, s, :],
                            in1=wz.to_broadcast([P, m, 2]), op=A.mult)
                        nc.vector.tensor_tensor(
                            out=zt[:, s, :], in0=gf[:, pr * m:(pr + 1) * m, 0:2],
                            in1=zd[:, s, :], op=A.add)

                    # y-lerp: yt[i] = zt[i,0] + wy*(zt[i,1] - zt[i,0])
                    yt = gpool.tile([P, 2 * m, 2], f32, tag="yt")
                    wy = t[:, :, 1:2]
                    for i in range(2):
                        a = slice(2 * i * m, (2 * i + 1) * m)
                        b = slice((2 * i + 1) * m, (2 * i + 2) * m)
                        o = slice(i * m, (i + 1) * m)
                        nc.vector.tensor_tensor(out=yt[:, o, :], in0=zt[:, b, :],
                                                in1=zt[:, a, :], op=A.subtract)
                        nc.vector.tensor_tensor(
                            out=yt[:, o, :], in0=yt[:, o, :],
                            in1=wy.to_broadcast([P, m, 2]), op=A.mult)
                        nc.vector.tensor_tensor(out=yt[:, o, :], in0=yt[:, o, :],
                                                in1=zt[:, a, :], op=A.add)

                    # x-lerp: out_l = yt[0] + wx*(yt[1] - yt[0]) -> bf16 pack
                    xd = wpool.tile([P, m, 2], f32, tag="xd")
                    wx = t[:, :, 0:1]
                    nc.vector.tensor_tensor(out=xd[:], in0=yt[:, m:2 * m, :],
                                            in1=yt[:, 0:m, :], op=A.subtract)
                    nc.vector.tensor_tensor(out=xd[:], in0=xd[:],
                                            in1=wx.to_broadcast([P, m, 2]),
                                            op=A.mult)
                    nc.vector.tensor_tensor(out=ot[:, :, 2 * l:2 * l + 2],
                                            in0=yt[:, 0:m, :], in1=xd[:],
                                            op=A.add)

                nc.sync.dma_start(out=orr[:, it * m:(it + 1) * m, :],
                                  in_=ot[:])

    nc.compile()
    return nc


def _get_compiled():
    global _COMPILED
    if _COMPILED is None:
        _COMPILED = _compile()
    return _COMPILED


def kernel(x: np.ndarray, tables: np.ndarray, _want_trace: bool = False):
    nc = _get_compiled()
    x = np.ascontiguousarray(np.asarray(x, dtype=np.float32))
    tables = np.asarray(tables, dtype=np.float32)
    vt = _build_packed_table(tables)
    consts = np.empty((P, 1, 3 * N_LEVELS_USED), np.float32)
    for l in range(N_LEVELS_USED):
        consts[:, :, 3 * l:3 * l + 3] = [float(VD[l] * VD[l]), float(VD[l]), 1.0]
    n_use = NPC * N_CORES
    in_maps = [{"x": x[c * NPC:(c + 1) * NPC], "vtab": vt, "consts": consts}
               for c in range(N_CORES)]
    res = run_bass_kernel_spmd(nc, in_maps, list(range(N_CORES)),
                               trace=_want_trace)
    out = np.empty((n_use, 16), dtype=np.float32)
    for c in range(N_CORES):
        out[c * NPC:(c + 1) * NPC] = res.results[c]["out"].astype(np.float32)
    if _want_trace:
        return out, res
    return out


# revision 12
# speedup vs baseline: 5.3027x; 1.6921x over previous
"""HashEmbedder (HashNeRF multires hash encoding) Trainium2 kernel.

The graded metric is the warm wall-clock of a full kernel() call, which under
the axon tunnel (~20-50 MB/s h2d+d2h) is dominated by bytes moved, not device
compute. Strategy:

 - Only levels 0..7 survive the reference's crop to 16 output columns.
 - POINT-sharded across the 8 NeuronCores: core c handles points
   [c*131072, (c+1)*131072) for all 8 levels, so x is shipped once (12 MB
   total) instead of replicated 8x.
 - Per level, the hash table is collapsed host-side into a DENSE VERTEX grid
   D_l[(R+1)^3, 2] = table_l[hash(vx,vy,vz)] (input-independent hash grids are
   precomputed once).  All 8 levels pack into one [TOTAL_VERTS, 2] bf16 buffer
   (~4.3 MB per core).  Layout [vx][vy][vz][feat] makes the two z-corners of a
   voxel contiguous (8 bytes), so each of the 4 (x,y) corner pairs is one
   gather row - no corner duplication, no hashing on device.
 - Outputs are written bf16 (32 MB total instead of 64) and upcast on host.
 - Device kernel: floor/frac in f32 (exact: values < 2^24), vertex index
   arithmetic in f32, 4 multi-offset indirect-DMA gathers per (chunk, level),
   trilinear lerp cascade in f32, pack to bf16.
"""
import os
import sys
import numpy as np

sys.path.insert(0, "/opt/trn_rl_repo")

import concourse.bass as bass
import concourse.tile as tile
from concourse import bacc, mybir
from concourse.bass_utils import run_bass_kernel_spmd
from contextlib import ExitStack

import ml_dtypes

BF16 = ml_dtypes.bfloat16

# ---- problem constants (hardcoded; kernel.py must be self-contained) ----
N_POINTS = 1048576
LOG2_T = 19
TABLE_SIZE = 1 << LOG2_T
BASE_RES = 16.0
FINEST_RES = 512.0
N_LEVELS_TOTAL = 16
N_LEVELS_USED = 8
N_CORES = 8

_b = np.exp((np.log(FINEST_RES) - np.log(BASE_RES)) / (N_LEVELS_TOTAL - 1))
RES = [int(np.floor(np.float32(BASE_RES) * np.float32(_b) ** np.float32(l)))
       for l in range(N_LEVELS_USED)]  # [16, 20, 25, 32, 40, 50, 64, 80]
VD = [r + 1 for r in RES]              # vertex grid dim per axis
NVERT = [d ** 3 for d in VD]
VBASE = [int(x) for x in np.cumsum([0] + NVERT)[:N_LEVELS_USED]]
TOTAL_VERTS = sum(NVERT)               # 1,075,325
VSHARD = (TOTAL_VERTS + N_CORES - 1) // N_CORES  # table rows shipped per core
TOTAL_PAD = VSHARD * N_CORES
XSCALE = 65536.0                       # x fixed-point uint16 encoding

P = 128
NPC = int(os.environ.get("BASSK_NPC", N_POINTS // N_CORES))  # points per core
PPP = NPC // P                         # points per partition per core
CHUNK = min(256, PPP)                  # points per partition per iteration
JB = int(os.environ.get("BASSK_JB", 32))  # gather offsets per indirect-DMA instr

_PRIMES = np.array([1, 2654435761, 805459861], dtype=np.uint64)

_COMPILED = None
_HGRIDS = None


def _hash_grids():
    """Input-independent per-level hash index grids H_l[(R+1)^3] int32."""
    global _HGRIDS
    if _HGRIDS is None:
        gs = []
        for l in range(N_LEVELS_USED):
            D = VD[l]
            vs = np.arange(D, dtype=np.uint64)
            hx = (vs * _PRIMES[0])[:, None, None]
            hy = (vs * _PRIMES[1])[None, :, None]
            hz = (vs * _PRIMES[2])[None, None, :]
            h = (hx ^ hy ^ hz) & np.uint64(TABLE_SIZE - 1)
            gs.append(h.astype(np.int32).ravel())
        _HGRIDS = gs
    return _HGRIDS


def _build_packed_table(tables: np.ndarray) -> np.ndarray:
    """[TOTAL_PAD, 2] bf16: all levels' dense vertex grids, x-major."""
    grids = _hash_grids()
    packed = np.zeros((TOTAL_PAD, 2), dtype=np.float32)
    for l in range(N_LEVELS_USED):
        packed[VBASE[l]:VBASE[l] + NVERT[l]] = tables[l][grids[l]]
    return packed.astype(BF16)


def _compile():
    nc = bacc.Bacc("TRN2", target_bir_lowering=False, debug=False,
                   num_devices=N_CORES)
    x_d = nc.dram_tensor("x", [NPC, 3], mybir.dt.uint16,
                         kind="ExternalInput").ap()
    v_d = nc.dram_tensor("vtab", [VSHARD, 2], mybir.dt.bfloat16,
                         kind="ExternalInput").ap()
    c_d = nc.dram_tensor("consts", [P, 1, 3 * N_LEVELS_USED], mybir.dt.float32,
                         kind="ExternalInput").ap()
    o_d = nc.dram_tensor("out", [NPC, 16], mybir.dt.bfloat16,
                         kind="ExternalOutput").ap()

    xr = x_d.rearrange("(p n) d -> p n d", p=P)   # [128, PPP, 3]
    orr = o_d.rearrange("(p n) d -> p n d", p=P)  # [128, PPP, 16]

    f32 = mybir.dt.float32
    i32 = mybir.dt.int32
    u16 = mybir.dt.uint16
    bf16 = mybir.dt.bfloat16
    A = mybir.AluOpType

    with tile.TileContext(nc) as tc:
        with ExitStack() as ctx:
            dpool = ctx.enter_context(tc.tile_pool(name="dram", bufs=1,
                                                   space="DRAM"))
            cpool = ctx.enter_context(tc.tile_pool(name="consts", bufs=1))
            xpool = ctx.enter_context(tc.tile_pool(name="x", bufs=2))
            opool = ctx.enter_context(tc.tile_pool(name="o", bufs=2))
            gpool = ctx.enter_context(tc.tile_pool(name="g", bufs=2))
            wpool = ctx.enter_context(tc.tile_pool(name="w", bufs=2))

            # all-gather the 1/8 table shard into the full packed table
            vsh = dpool.tile([VSHARD, 2], bf16, tag="vsh")
            vfull = dpool.tile([TOTAL_PAD, 2], bf16, tag="vfull")
            nc.gpsimd.dma_start(out=vsh[:], in_=v_d[:])
            nc.gpsimd.collective_compute(
                "AllGather", A.bypass,
                replica_groups=[list(range(N_CORES))],
                ins=[vsh[:].opt()], outs=[vfull[:].opt()],
            )

            ct = cpool.tile([P, 1, 3 * N_LEVELS_USED], f32)
            nc.sync.dma_start(out=ct[:], in_=c_d[:])

            m = CHUNK
            for it in range(PPP // m):
                xu = xpool.tile([P, m, 3], u16, tag="xu")
                nc.sync.dma_start(out=xu[:], in_=xr[:, it * m:(it + 1) * m, :])
                xt = xpool.tile([P, m, 3], f32, tag="xt")
                nc.scalar.copy(out=xt[:], in_=xu[:])  # exact ints < 2^16
                ot = opool.tile([P, m, 16], bf16, tag="ot")

                for l in range(N_LEVELS_USED):
                    c3 = ct[:, :, 3 * l:3 * l + 3]   # [S1, S2, 1]
                    t = wpool.tile([P, m, 3], f32, tag="t")
                    nc.vector.tensor_scalar_mul(out=t[:], in0=xt[:],
                                                scalar1=float(RES[l]) / XSCALE)
                    ti = wpool.tile([P, m, 3], i32, tag="ti")
                    nc.scalar.copy(out=ti[:], in_=t[:])    # round-to-nearest
                    bf = wpool.tile([P, m, 3], f32, tag="bf")
                    nc.scalar.copy(out=bf[:], in_=ti[:])
                    fx = wpool.tile([P, m, 3], f32, tag="fx")
                    nc.vector.tensor_tensor(out=fx[:], in0=bf[:], in1=t[:],
                                            op=A.is_gt)   # 1.0 where rounded up
                    nc.vector.tensor_tensor(out=bf[:], in0=bf[:], in1=fx[:],
                                            op=A.subtract)  # bf = floor(t)
                    nc.vector.tensor_tensor(out=t[:], in0=t[:], in1=bf[:],
                                            op=A.subtract)  # t = frac weights
                    nc.vector.tensor_tensor(out=fx[:], in0=bf[:],
                                            in1=c3.to_broadcast([P, m, 3]),
                                            op=A.mult)
                    vertf = wpool.tile([P, m, 1], f32, tag="vertf")
                    nc.vector.tensor_reduce(out=vertf[:], in_=fx[:],
                                            axis=mybir.AxisListType.X, op=A.add)

                    # 4 corner-pair index tiles: vert + base + i*S1 + j*S2
                    S1, S2 = VD[l] * VD[l], VD[l]
                    gp = gpool.tile([P, 4 * m, 4], bf16, tag="gp")
                    for pair in range(4):
                        i, j = pair >> 1, pair & 1
                        off = float(VBASE[l] + i * S1 + j * S2)
                        vp = wpool.tile([P, m, 1], f32, tag=f"vp{pair}")
                        nc.vector.tensor_scalar_add(out=vp[:], in0=vertf[:],
                                                    scalar1=off)
                        vi = wpool.tile([P, m, 1], i32, tag=f"vi{pair}")
                        nc.scalar.copy(out=vi[:], in_=vp[:])  # exact ints
                        jb = min(JB, m)
                        if jb == 1:
                            for j0 in range(m):
                                nc.gpsimd.indirect_dma_start(
                                    out=gp[:, pair * m + j0, :],
                                    out_offset=None,
                                    in_=vfull[:],
                                    in_offset=bass.IndirectOffsetOnAxis(
                                        ap=vi[:, j0, :], axis=0),
                                )
                        else:
                            for j0 in range(0, m, jb):
                                nc.gpsimd.indirect_dma_start(
                                    out=gp[:, pair * m + j0:pair * m + j0 + jb, :],
                                    out_offset=None,
                                    in_=vfull[:],
                                    in_offset=bass.IndirectOffsetOnAxis(
                                        ap=vi[:, j0:j0 + jb, 0:1], axis=0),
                                )

                    gf = gpool.tile([P, 4 * m, 4], f32, tag="gf")
                    nc.vector.tensor_copy(out=gf[:], in_=gp[:])

                    # z-lerp within each pair: zt[p] = g0 + wz*(g1 - g0)
                    zd = gpool.tile([P, 4 * m, 2], f32, tag="zd")
                    nc.vector.tensor_tensor(out=zd[:], in0=gf[:, :, 2:4],
                                            in1=gf[:, :, 0:2], op=A.subtract)
                    zt = gpool.tile([P, 4 * m, 2], f32, tag="zt")
                    wz = t[:, :, 2:3]
                    for pr in range(4):
                        s = slice(pr * m, (pr + 1) * m)
                        nc.vector.tensor_tensor(
                            out=zd[:, s, :], in0=zd[:, s, :],
                            in1=wz.to_broadcast([P, m, 2]), op=A.mult)
                        nc.vector.tensor_tensor(
                            out=zt[:, s, :], in0=gf[:, pr * m:(pr + 1) * m, 0:2],
                            in1=zd[:, s, :], op=A.add)

                    # y-lerp: yt[i] = zt[i,0] + wy*(zt[i,1] - zt[i,0])
                    yt = gpool.tile([P, 2 * m, 2], f32, tag="yt")
                    wy = t[:, :, 1:2]
                    for i in range(2):
                        a = slice(2 * i * m, (2 * i + 1) * m)
                        b = slice((2 * i + 1) * m, (2 * i + 2) * m)
                        o = slice(i * m, (i + 1) * m)
                        nc.vector.tensor_tensor(out=yt[:, o, :], in0=zt[:, b, :],
                                                in1=zt[:, a, :], op=A.subtract)
                        nc.vector.tensor_tensor(
                            out=yt[:, o, :], in0=yt[:, o, :],
                            in1=wy.to_broadcast([P, m, 2]), op=A.mult)
                        nc.vector.tensor_tensor(out=yt[:, o, :], in0=yt[:, o, :],
                                                in1=zt[:, a, :], op=A.add)

                    # x-lerp: out_l = yt[0] + wx*(yt[1] - yt[0]) -> bf16 pack
                    xd = wpool.tile([P, m, 2], f32, tag="xd")
                    wx = t[:, :, 0:1]
                    nc.vector.tensor_tensor(out=xd[:], in0=yt[:, m:2 * m, :],
                                            in1=yt[:, 0:m, :], op=A.subtract)
                    nc.vector.tensor_tensor(out=xd[:], in0=xd[:],
                                            in1=wx.to_broadcast([P, m, 2]),
                                            op=A.mult)
                    nc.vector.tensor_tensor(out=ot[:, :, 2 * l:2 * l + 2],
                                            in0=yt[:, 0:m, :], in1=xd[:],
                                            op=A.add)

                nc.sync.dma_start(out=orr[:, it * m:(it + 1) * m, :],
                                  in_=ot[:])

    nc.compile()
    return nc


def _get_compiled():
    global _COMPILED
    if _COMPILED is None:
        _COMPILED = _compile()
    return _COMPILED


def kernel(x: np.ndarray, tables: np.ndarray, _want_trace: bool = False):
    nc = _get_compiled()
    x = np.asarray(x, dtype=np.float32)
    tables = np.asarray(tables, dtype=np.float32)
    # uint16 fixed-point encode: xq/65536 is within 1.6e-5 of x, and the
    # trilinear interpolant is continuous in x, so the output perturbation is
    # far below the bf16 table quantization already present.
    xq = np.ascontiguousarray((x * np.float32(XSCALE)).astype(np.uint16))
    vt = _build_packed_table(tables)
    consts = np.empty((P, 1, 3 * N_LEVELS_USED), np.float32)
    for l in range(N_LEVELS_USED):
        consts[:, :, 3 * l:3 * l + 3] = [float(VD[l] * VD[l]), float(VD[l]), 1.0]
    n_use = NPC * N_CORES
    in_maps = [{"x": xq[c * NPC:(c + 1) * NPC],
                "vtab": vt[c * VSHARD:(c + 1) * VSHARD], "consts": consts}
               for c in range(N_CORES)]
    res = run_bass_kernel_spmd(nc, in_maps, list(range(N_CORES)),
                               trace=_want_trace)
    out = np.empty((n_use, 16), dtype=np.float32)
    for c in range(N_CORES):
        out[c * NPC:(c + 1) * NPC] = res.results[c]["out"].astype(np.float32)
    if _want_trace:
        return out, res
    return out


# revision 30
# speedup vs baseline: 8.0185x; 1.5122x over previous
"""HashEmbedder (HashNeRF multires hash encoding) Trainium2 kernel.

The graded metric is the warm wall-clock of a full kernel() call. Under the
axon tunnel (~20-80 MB/s, high variance) that is dominated by (a) bytes moved
host<->device and (b) indirect-DMA instruction count (~100us each). Strategy:

 - Only levels 0..7 survive the reference's crop to 16 output columns.
 - POINT-sharded across the 8 NeuronCores: core c handles points
   [c*131072, (c+1)*131072) for all 8 levels, so x is shipped once, encoded
   uint16 fixed-point (6 MB total; the interpolant is continuous in x so the
   1.6e-5 perturbation is negligible vs the 2e-2 gate).
 - Per level, the hash table is collapsed host-side into a DENSE VERTEX grid
   D_l[(R+1)^3, 2] = table_l[hash(vx,vy,vz)] (input-independent hash grids
   precomputed once), packed over levels into one [TOTAL_PAD, 2] bf16 buffer
   (4.3 MB).  Each core is shipped a 1/8 shard and an on-device AllGather
   rebuilds the full table (bytes over the tunnel beat NeuronLink traffic).
 - Outputs are written bf16 (32 MB total instead of 64) and upcast on host.
   (int8 outputs were tried and rejected: ~1.5e-2 rel err vs the 2e-2 gate.)
 - Device kernel: floor/frac in f32 (exact: values < 2^24), vertex index
   arithmetic in f32, trilinear lerp cascade in f32, pack to bf16.
 - Gathers: the DGE supports ONE offset per partition per indirect DMA and
   fetches dest-partition-size contiguous bytes from it (measured; the
   multi-offset form the simulator accepts silently misbehaves on HW). With
   table layout [vx][vy][vz][feat], the (S2+2)-row span starting at corner
   (i,0,0) covers all four (j,k) corners at static offsets j*S2+k, so one
   span gather per (point, i) replaces four 8-byte pair gathers: 16384
   instructions/core instead of 32768.
"""
import os
import sys
import numpy as np

sys.path.insert(0, "/opt/trn_rl_repo")

import concourse.bass as bass
import concourse.tile as tile
from concourse import bacc, mybir
from concourse.bass_utils import run_bass_kernel_spmd
from contextlib import ExitStack

import ml_dtypes

BF16 = ml_dtypes.bfloat16

# ---- problem constants (hardcoded; kernel.py must be self-contained) ----
N_POINTS = 1048576
LOG2_T = 19
TABLE_SIZE = 1 << LOG2_T
BASE_RES = 16.0
FINEST_RES = 512.0
N_LEVELS_TOTAL = 16
N_LEVELS_USED = 8
N_CORES = 8

_b = np.exp((np.log(FINEST_RES) - np.log(BASE_RES)) / (N_LEVELS_TOTAL - 1))
RES = [int(np.floor(np.float32(BASE_RES) * np.float32(_b) ** np.float32(l)))
       for l in range(N_LEVELS_USED)]  # [16, 20, 25, 32, 40, 50, 64, 80]
VD = [r + 1 for r in RES]              # vertex grid dim per axis
NVERT = [d ** 3 for d in VD]
VBASE = [int(x) for x in np.cumsum([0] + NVERT)[:N_LEVELS_USED]]
TOTAL_VERTS = sum(NVERT)               # 1,075,325
VSHARD = (TOTAL_VERTS + N_CORES - 1) // N_CORES  # table rows shipped per core
TOTAL_PAD = VSHARD * N_CORES
XSCALE = 65536.0                       # x fixed-point uint16 encoding
# int8 output encoding: outputs are convex combinations of table values, so
# |out| <= 1e-4*(1+2^-9) (bf16-rounded tables). Scale 126/1e-4 keeps the
# rounded magnitude <= 127 and the quantization step adds ~0.8% rel-norm
# error -- far under the 2e-2 gate.
OSCALE = 126.0 / 1e-4
OUT8 = os.environ.get("BASSK_OUT8", "0") != "0"  # rejected: ~1.5e-2 rel err
AG = os.environ.get("BASSK_AG", "1") != "0"  # all-gather table vs replicate
NOGATHER = os.environ.get("BASSK_NOGATHER", "0") != "0"  # timing probe only
# span gathers: one indirect DMA per (point, x-corner) fetches the whole
# contiguous (S2+2)-row span covering both y-corner pairs (the DGE gathers
# dest-partition-size bytes from one offset), halving gather instructions.
SPAN = os.environ.get("BASSK_SPAN", "1") != "0"
SM = 64                                 # points per span-gather sub-block
MAXSP = max(VD) + 2                     # 83 rows; span tile sized for level 7

P = 128
NPC = int(os.environ.get("BASSK_NPC", N_POINTS // N_CORES))  # points per core
PPP = NPC // P                         # points per partition per core
CHUNK = min(256, PPP)                  # points per partition per iteration
# NOTE: the DGE ucode supports exactly ONE offset per partition per indirect
# DMA: it gathers dest-partition-size contiguous bytes from offset[p, 0] and
# ignores further offsets (verified empirically).  Gathers must stay at one
# point-column per instruction.
JB = int(os.environ.get("BASSK_JB", 1))

_PRIMES = np.array([1, 2654435761, 805459861], dtype=np.uint64)

_COMPILED = None
_HGRIDS = None


def _hash_grids():
    """Input-independent per-level hash index grids H_l[(R+1)^3] int32."""
    global _HGRIDS
    if _HGRIDS is None:
        gs = []
        for l in range(N_LEVELS_USED):
            D = VD[l]
            vs = np.arange(D, dtype=np.uint64)
            hx = (vs * _PRIMES[0])[:, None, None]
            hy = (vs * _PRIMES[1])[None, :, None]
            hz = (vs * _PRIMES[2])[None, None, :]
            h = (hx ^ hy ^ hz) & np.uint64(TABLE_SIZE - 1)
            gs.append(h.astype(np.int32).ravel())
        _HGRIDS = gs
    return _HGRIDS


def _build_packed_table(tables: np.ndarray) -> np.ndarray:
    """[TOTAL_PAD, 2] bf16: all levels' dense vertex grids, x-major."""
    grids = _hash_grids()
    packed = np.zeros((TOTAL_PAD, 2), dtype=np.float32)
    for l in range(N_LEVELS_USED):
        packed[VBASE[l]:VBASE[l] + NVERT[l]] = tables[l][grids[l]]
    return packed.astype(BF16)


def _compile():
    nc = bacc.Bacc("TRN2", target_bir_lowering=False, debug=False,
                   num_devices=N_CORES)
    x_d = nc.dram_tensor("x", [NPC, 3], mybir.dt.uint16,
                         kind="ExternalInput").ap()
    v_d = nc.dram_tensor("vtab", [VSHARD if AG else TOTAL_PAD, 2],
                         mybir.dt.bfloat16, kind="ExternalInput").ap()
    c_d = nc.dram_tensor("consts", [P, 1, 3 * N_LEVELS_USED], mybir.dt.float32,
                         kind="ExternalInput").ap()
    o_d = nc.dram_tensor("out", [NPC, 16],
                         mybir.dt.int8 if OUT8 else mybir.dt.bfloat16,
                         kind="ExternalOutput").ap()

    xr = x_d.rearrange("(p n) d -> p n d", p=P)   # [128, PPP, 3]
    orr = o_d.rearrange("(p n) d -> p n d", p=P)  # [128, PPP, 16]

    f32 = mybir.dt.float32
    i32 = mybir.dt.int32
    u16 = mybir.dt.uint16
    bf16 = mybir.dt.bfloat16
    A = mybir.AluOpType

    with tile.TileContext(nc) as tc:
        with ExitStack() as ctx:
            dpool = ctx.enter_context(tc.tile_pool(name="dram", bufs=1,
                                                   space="DRAM"))
            cpool = ctx.enter_context(tc.tile_pool(name="consts", bufs=1))
            xpool = ctx.enter_context(tc.tile_pool(name="x", bufs=2))
            opool = ctx.enter_context(tc.tile_pool(name="o", bufs=2))
            gpool = ctx.enter_context(tc.tile_pool(name="g", bufs=1 if SPAN
                                                   else 2))
            wpool = ctx.enter_context(tc.tile_pool(name="w", bufs=2))
            if SPAN:
                spool = ctx.enter_context(tc.tile_pool(name="s", bufs=2))

            if AG:
                # all-gather the 1/8 table shard into the full packed table
                vsh = dpool.tile([VSHARD, 2], bf16, tag="vsh")
                vfull = dpool.tile([TOTAL_PAD, 2], bf16, tag="vfull")
                nc.gpsimd.dma_start(out=vsh[:], in_=v_d[:])
                nc.gpsimd.collective_compute(
                    "AllGather", A.bypass,
                    replica_groups=[list(range(N_CORES))],
                    ins=[vsh[:].opt()], outs=[vfull[:].opt()],
                )
            else:
                vfull = v_d

            ct = cpool.tile([P, 1, 3 * N_LEVELS_USED], f32)
            nc.sync.dma_start(out=ct[:], in_=c_d[:])

            m = CHUNK
            for it in range(PPP // m):
                xu = xpool.tile([P, m, 3], u16, tag="xu")
                nc.sync.dma_start(out=xu[:], in_=xr[:, it * m:(it + 1) * m, :])
                xt = xpool.tile([P, m, 3], f32, tag="xt")
                nc.scalar.copy(out=xt[:], in_=xu[:])  # exact ints < 2^16
                ot = opool.tile([P, m, 16], mybir.dt.int8 if OUT8 else bf16,
                                tag="ot")

                for l in range(N_LEVELS_USED):
                    c3 = ct[:, :, 3 * l:3 * l + 3]   # [S1, S2, 1]
                    t = wpool.tile([P, m, 3], f32, tag="t")
                    nc.vector.tensor_scalar_mul(out=t[:], in0=xt[:],
                                                scalar1=float(RES[l]) / XSCALE)
                    ti = wpool.tile([P, m, 3], i32, tag="ti")
                    nc.scalar.copy(out=ti[:], in_=t[:])    # round-to-nearest
                    bf = wpool.tile([P, m, 3], f32, tag="bf")
                    nc.scalar.copy(out=bf[:], in_=ti[:])
                    fx = wpool.tile([P, m, 3], f32, tag="fx")
                    nc.vector.tensor_tensor(out=fx[:], in0=bf[:], in1=t[:],
                                            op=A.is_gt)   # 1.0 where rounded up
                    nc.vector.tensor_tensor(out=bf[:], in0=bf[:], in1=fx[:],
                                            op=A.subtract)  # bf = floor(t)
                    nc.vector.tensor_tensor(out=t[:], in0=t[:], in1=bf[:],
                                            op=A.subtract)  # t = frac weights
                    nc.vector.tensor_tensor(out=fx[:], in0=bf[:],
                                            in1=c3.to_broadcast([P, m, 3]),
                                            op=A.mult)
                    vertf = wpool.tile([P, m, 1], f32, tag="vertf")
                    nc.vector.tensor_reduce(out=vertf[:], in_=fx[:],
                                            axis=mybir.AxisListType.X, op=A.add)

                    S1, S2 = VD[l] * VD[l], VD[l]
                    gf = gpool.tile([P, 4 * m, 4], f32, tag="gf")
                    if SPAN:
                        # one gather per (point, i): the (S2+2)-row span
                        # starting at vert + i*S1 covers corners (i, j, k)
                        # for j,k in {0,1} at static row offsets j*S2 + k.
                        sp2 = (S2 + 2) * 2
                        vsi = []
                        for i in range(2):
                            vp = wpool.tile([P, m, 1], f32, tag=f"vp{i}")
                            nc.vector.tensor_scalar_add(
                                out=vp[:], in0=vertf[:],
                                scalar1=float(VBASE[l] + i * S1))
                            vi = wpool.tile([P, m, 1], i32, tag=f"vi{i}")
                            nc.scalar.copy(out=vi[:], in_=vp[:])
                            vsi.append(vi)
                        sm = min(SM, m)
                        for sub in range(m // sm):
                            gs = spool.tile([P, sm, 2, MAXSP * 2], bf16,
                                            tag="gs")
                            if NOGATHER:
                                nc.vector.memset(gs[:, :, :, :sp2], 0.0)
                            else:
                                for j0 in range(sm):
                                    for i in range(2):
                                        nc.gpsimd.indirect_dma_start(
                                            out=gs[:, j0, i, :sp2],
                                            out_offset=None,
                                            in_=vfull[:],
                                            in_offset=bass.IndirectOffsetOnAxis(
                                                ap=vsi[i][:, sub * sm + j0, :],
                                                axis=0),
                                        )
                            for pair in range(4):
                                i, j = pair >> 1, pair & 1
                                a = pair * m + sub * sm
                                nc.scalar.copy(
                                    out=gf[:, a:a + sm, :],
                                    in_=gs[:, :, i, 2 * j * S2:2 * j * S2 + 4])
                    else:
                        # 4 corner-pair index tiles: vert + base + i*S1 + j*S2
                        gp = gpool.tile([P, 4 * m, 4], bf16, tag="gp")
                        for pair in range(4):
                            i, j = pair >> 1, pair & 1
                            off = float(VBASE[l] + i * S1 + j * S2)
                            vp = wpool.tile([P, m, 1], f32, tag=f"vp{pair}")
                            nc.vector.tensor_scalar_add(out=vp[:], in0=vertf[:],
                                                        scalar1=off)
                            vi = wpool.tile([P, m, 1], i32, tag=f"vi{pair}")
                            nc.scalar.copy(out=vi[:], in_=vp[:])  # exact ints
                            if NOGATHER:
                                if pair == 0:
                                    nc.vector.memset(gp[:], 0.0)
                            else:
                                for j0 in range(m):
                                    nc.gpsimd.indirect_dma_start(
                                        out=gp[:, pair * m + j0, :],
                                        out_offset=None,
                                        in_=vfull[:],
                                        in_offset=bass.IndirectOffsetOnAxis(
                                            ap=vi[:, j0, :], axis=0),
                                    )
                        nc.vector.tensor_copy(out=gf[:], in_=gp[:])

                    # z-lerp within each pair: zt[p] = g0 + wz*(g1 - g0)
                    zd = gpool.tile([P, 4 * m, 2], f32, tag="zd")
                    nc.vector.tensor_tensor(out=zd[:], in0=gf[:, :, 2:4],
                                            in1=gf[:, :, 0:2], op=A.subtract)
                    zt = gpool.tile([P, 4 * m, 2], f32, tag="zt")
                    wz = t[:, :, 2:3]
                    for pr in range(4):
                        s = slice(pr * m, (pr + 1) * m)
                        nc.vector.tensor_tensor(
                            out=zd[:, s, :], in0=zd[:, s, :],
                            in1=wz.to_broadcast([P, m, 2]), op=A.mult)
                        nc.vector.tensor_tensor(
                            out=zt[:, s, :], in0=gf[:, pr * m:(pr + 1) * m, 0:2],
                            in1=zd[:, s, :], op=A.add)

                    # y-lerp: yt[i] = zt[i,0] + wy*(zt[i,1] - zt[i,0])
                    yt = gpool.tile([P, 2 * m, 2], f32, tag="yt")
                    wy = t[:, :, 1:2]
                    for i in range(2):
                        a = slice(2 * i * m, (2 * i + 1) * m)
                        b = slice((2 * i + 1) * m, (2 * i + 2) * m)
                        o = slice(i * m, (i + 1) * m)
                        nc.vector.tensor_tensor(out=yt[:, o, :], in0=zt[:, b, :],
                                                in1=zt[:, a, :], op=A.subtract)
                        nc.vector.tensor_tensor(
                            out=yt[:, o, :], in0=yt[:, o, :],
                            in1=wy.to_broadcast([P, m, 2]), op=A.mult)
                        nc.vector.tensor_tensor(out=yt[:, o, :], in0=yt[:, o, :],
                                                in1=zt[:, a, :], op=A.add)

                    # x-lerp: out_l = yt[0] + wx*(yt[1] - yt[0])
                    xd = wpool.tile([P, m, 2], f32, tag="xd")
                    wx = t[:, :, 0:1]
                    nc.vector.tensor_tensor(out=xd[:], in0=yt[:, m:2 * m, :],
                                            in1=yt[:, 0:m, :], op=A.subtract)
                    nc.vector.tensor_tensor(out=xd[:], in0=xd[:],
                                            in1=wx.to_broadcast([P, m, 2]),
                                            op=A.mult)
                    if OUT8:
                        xo = wpool.tile([P, m, 2], f32, tag="xo")
                        nc.vector.tensor_tensor(out=xo[:], in0=yt[:, 0:m, :],
                                                in1=xd[:], op=A.add)
                        nc.vector.tensor_scalar_mul(out=xo[:], in0=xo[:],
                                                    scalar1=OSCALE)
                        qi = wpool.tile([P, m, 2], i32, tag="qi")
                        nc.scalar.copy(out=qi[:], in_=xo[:])  # RNE, in [-127,127]
                        nc.scalar.copy(out=ot[:, :, 2 * l:2 * l + 2], in_=qi[:])
                    else:
                        nc.vector.tensor_tensor(out=ot[:, :, 2 * l:2 * l + 2],
                                                in0=yt[:, 0:m, :], in1=xd[:],
                                                op=A.add)

                nc.sync.dma_start(out=orr[:, it * m:(it + 1) * m, :],
                                  in_=ot[:])

    nc.compile()
    return nc


def _get_compiled():
    global _COMPILED
    if _COMPILED is None:
        _COMPILED = _compile()
    return _COMPILED


def kernel(x: np.ndarray, tables: np.ndarray, _want_trace: bool = False):
    nc = _get_compiled()
    x = np.asarray(x, dtype=np.float32)
    tables = np.asarray(tables, dtype=np.float32)
    # uint16 fixed-point encode: xq/65536 is within 1.6e-5 of x, and the
    # trilinear interpolant is continuous in x, so the output perturbation is
    # far below the bf16 table quantization already present.
    xq = np.ascontiguousarray((x * np.float32(XSCALE)).astype(np.uint16))
    vt = _build_packed_table(tables)
    consts = np.empty((P, 1, 3 * N_LEVELS_USED), np.float32)
    for l in range(N_LEVELS_USED):
        consts[:, :, 3 * l:3 * l + 3] = [float(VD[l] * VD[l]), float(VD[l]), 1.0]
    n_use = NPC * N_CORES
    in_maps = [{"x": xq[c * NPC:(c + 1) * NPC],
                "vtab": vt[c * VSHARD:(c + 1) * VSHARD] if AG else vt,
                "consts": consts}
               for c in range(N_CORES)]
    res = run_bass_kernel_spmd(nc, in_maps, list(range(N_CORES)),
                               trace=_want_trace)
    out = np.empty((n_use, 16), dtype=np.float32)
    for c in range(N_CORES):
        o = res.results[c]["out"].astype(np.float32)
        if OUT8:
            o *= np.float32(1.0 / OSCALE)
        out[c * NPC:(c + 1) * NPC] = o
    if _want_trace:
        return out, res
    return out


# revision 40
# speedup vs baseline: 8.1084x; 1.0112x over previous
"""HashEmbedder (HashNeRF multires hash encoding) Trainium2 kernel.

The graded metric is the warm wall-clock of a full kernel() call. Under the
axon tunnel (~20-80 MB/s, high variance) that is dominated by (a) bytes moved
host<->device and (b) indirect-DMA instruction count (~100us each). Strategy:

 - Only levels 0..7 survive the reference's crop to 16 output columns.
 - POINT-sharded across the 8 NeuronCores: core c handles points
   [c*131072, (c+1)*131072) for all 8 levels, so x is shipped once, encoded
   uint16 fixed-point (6 MB total; the interpolant is continuous in x so the
   1.6e-5 perturbation is negligible vs the 2e-2 gate).
 - Per level, the hash table is collapsed host-side into a DENSE VERTEX grid
   D_l[(R+1)^3, 2] = table_l[hash(vx,vy,vz)] (input-independent hash grids
   precomputed once), packed over levels into one [TOTAL_PAD, 2] bf16 buffer
   (4.3 MB).  Each core is shipped a 1/8 shard and an on-device AllGather
   rebuilds the full table (bytes over the tunnel beat NeuronLink traffic).
 - Outputs are written bf16 (32 MB total instead of 64) and upcast on host.
   (int8 outputs were tried and rejected: ~1.5e-2 rel err vs the 2e-2 gate.)
 - Device kernel: floor/frac in f32 (exact: values < 2^24), vertex index
   arithmetic in f32, trilinear lerp cascade in f32, pack to bf16.
 - Gathers: the DGE supports ONE offset per partition per indirect DMA and
   fetches dest-partition-size contiguous bytes from it (measured; the
   multi-offset form the simulator accepts silently misbehaves on HW). With
   table layout [vx][vy][vz][feat], the (S2+2)-row span starting at corner
   (i,0,0) covers all four (j,k) corners at static offsets j*S2+k, so one
   span gather per (point, i) replaces four 8-byte pair gathers: 16384
   instructions/core instead of 32768.
"""
import os
import sys
import numpy as np

sys.path.insert(0, "/opt/trn_rl_repo")

import concourse.bass as bass
import concourse.tile as tile
from concourse import bacc, mybir
from concourse.bass_utils import run_bass_kernel_spmd
from contextlib import ExitStack

import ml_dtypes

BF16 = ml_dtypes.bfloat16

# ---- problem constants (hardcoded; kernel.py must be self-contained) ----
N_POINTS = 1048576
LOG2_T = 19
TABLE_SIZE = 1 << LOG2_T
BASE_RES = 16.0
FINEST_RES = 512.0
N_LEVELS_TOTAL = 16
N_LEVELS_USED = 8
N_CORES = 8

_b = np.exp((np.log(FINEST_RES) - np.log(BASE_RES)) / (N_LEVELS_TOTAL - 1))
RES = [int(np.floor(np.float32(BASE_RES) * np.float32(_b) ** np.float32(l)))
       for l in range(N_LEVELS_USED)]  # [16, 20, 25, 32, 40, 50, 64, 80]
VD = [r + 1 for r in RES]              # vertex grid dim per axis
NVERT = [d ** 3 for d in VD]
VBASE = [int(x) for x in np.cumsum([0] + NVERT)[:N_LEVELS_USED]]
TOTAL_VERTS = sum(NVERT)               # 1,075,325
VSHARD = (TOTAL_VERTS + N_CORES - 1) // N_CORES  # table rows shipped per core
TOTAL_PAD = VSHARD * N_CORES
XSCALE = 65536.0                       # x fixed-point uint16 encoding
# int8 output encoding: outputs are convex combinations of table values, so
# |out| <= 1e-4*(1+2^-9) (bf16-rounded tables). Scale 126/1e-4 keeps the
# rounded magnitude <= 127 and the quantization step adds ~0.8% rel-norm
# error -- far under the 2e-2 gate.
OSCALE = 126.0 / 1e-4
OUT8 = os.environ.get("BASSK_OUT8", "0") != "0"  # rejected: ~1.5e-2 rel err
AG = os.environ.get("BASSK_AG", "1") != "0"  # all-gather table vs replicate
NOGATHER = os.environ.get("BASSK_NOGATHER", "0") != "0"  # timing probe only
# span gathers: one indirect DMA per (point, x-corner) fetches the whole
# contiguous (S2+2)-row span covering both y-corner pairs (the DGE gathers
# dest-partition-size bytes from one offset), halving gather instructions.
SPAN = os.environ.get("BASSK_SPAN", "1") != "0"
SM = 64                                 # points per span-gather sub-block
MAXSP = max(VD) + 2                     # 83 rows; span tile sized for level 7
# full-voxel spans: ONE gather per (point, level) fetching the whole
# (S1+S2+2)-row block that covers all 8 corners at static offsets
# i*S1 + j*S2 + k.  Indirect-DMA cost is per-instruction (~70us), not
# per-byte, so fatter gathers win; sub-block size sm_l is set by SBUF.
# Measured gather cost ~= 67us/instruction + bytes/(~15 GB/s). Full-voxel
# spans (one gather per point-level) were tried for all levels (+0.1-0.4s:
# byte cost dominates at fine levels) and as a coarse-level-only mix (even to
# slightly worse: smaller SBUF sub-blocks offset the instruction savings), so
# the default stays pure 2-span.  BASSK_FULLSPAN: "1", "0", or "mixed".
_FS = os.environ.get("BASSK_FULLSPAN", "0")
FULL_L = [(_FS == "1" or (_FS == "mixed" and l <= 4))
          for l in range(N_LEVELS_USED)]
# per-level points per sub-block, sized so sm*(S1+S2+2)*4B <= ~23KB
FSM = [16, 8, 8, 4, 2, 2, 2, 1]
FSNEL = max(FSM[l] * (VD[l] * VD[l] + VD[l] + 2) * 2
            for l in range(N_LEVELS_USED) if FULL_L[l]) if any(FULL_L) else 0

P = 128
NPC = int(os.environ.get("BASSK_NPC", N_POINTS // N_CORES))  # points per core
PPP = NPC // P                         # points per partition per core
CHUNK = min(256, PPP)                  # points per partition per iteration
# NOTE: the DGE ucode supports exactly ONE offset per partition per indirect
# DMA: it gathers dest-partition-size contiguous bytes from offset[p, 0] and
# ignores further offsets (verified empirically).  Gathers must stay at one
# point-column per instruction.
JB = int(os.environ.get("BASSK_JB", 1))

_PRIMES = np.array([1, 2654435761, 805459861], dtype=np.uint64)

_COMPILED = None
_HGRIDS = None


def _hash_grids():
    """Input-independent per-level hash index grids H_l[(R+1)^3] int32."""
    global _HGRIDS
    if _HGRIDS is None:
        gs = []
        for l in range(N_LEVELS_USED):
            D = VD[l]
            vs = np.arange(D, dtype=np.uint64)
            hx = (vs * _PRIMES[0])[:, None, None]
            hy = (vs * _PRIMES[1])[None, :, None]
            hz = (vs * _PRIMES[2])[None, None, :]
            h = (hx ^ hy ^ hz) & np.uint64(TABLE_SIZE - 1)
            gs.append(h.astype(np.int32).ravel())
        _HGRIDS = gs
    return _HGRIDS


def _build_packed_table(tables: np.ndarray) -> np.ndarray:
    """[TOTAL_PAD, 2] bf16: all levels' dense vertex grids, x-major."""
    grids = _hash_grids()
    packed = np.zeros((TOTAL_PAD, 2), dtype=np.float32)
    for l in range(N_LEVELS_USED):
        packed[VBASE[l]:VBASE[l] + NVERT[l]] = tables[l][grids[l]]
    return packed.astype(BF16)


def _compile():
    nc = bacc.Bacc("TRN2", target_bir_lowering=False, debug=False,
                   num_devices=N_CORES)
    x_d = nc.dram_tensor("x", [NPC, 3], mybir.dt.uint16,
                         kind="ExternalInput").ap()
    v_d = nc.dram_tensor("vtab", [VSHARD if AG else TOTAL_PAD, 2],
                         mybir.dt.bfloat16, kind="ExternalInput").ap()
    c_d = nc.dram_tensor("consts", [P, 1, 3 * N_LEVELS_USED], mybir.dt.float32,
                         kind="ExternalInput").ap()
    o_d = nc.dram_tensor("out", [NPC, 16],
                         mybir.dt.int8 if OUT8 else mybir.dt.bfloat16,
                         kind="ExternalOutput").ap()

    xr = x_d.rearrange("(p n) d -> p n d", p=P)   # [128, PPP, 3]
    orr = o_d.rearrange("(p n) d -> p n d", p=P)  # [128, PPP, 16]

    f32 = mybir.dt.float32
    i32 = mybir.dt.int32
    u16 = mybir.dt.uint16
    bf16 = mybir.dt.bfloat16
    A = mybir.AluOpType

    with tile.TileContext(nc) as tc:
        with ExitStack() as ctx:
            dpool = ctx.enter_context(tc.tile_pool(name="dram", bufs=1,
                                                   space="DRAM"))
            cpool = ctx.enter_context(tc.tile_pool(name="consts", bufs=1))
            xpool = ctx.enter_context(tc.tile_pool(name="x", bufs=2))
            opool = ctx.enter_context(tc.tile_pool(name="o", bufs=2))
            gpool = ctx.enter_context(tc.tile_pool(name="g", bufs=1 if SPAN
                                                   else 2))
            wpool = ctx.enter_context(tc.tile_pool(name="w", bufs=2))
            if SPAN or any(FULL_L):
                spool = ctx.enter_context(tc.tile_pool(name="s", bufs=1))

            if AG:
                # all-gather the 1/8 table shard into the full packed table
                vsh = dpool.tile([VSHARD, 2], bf16, tag="vsh")
                vfull = dpool.tile([TOTAL_PAD, 2], bf16, tag="vfull")
                nc.gpsimd.dma_start(out=vsh[:], in_=v_d[:])
                nc.gpsimd.collective_compute(
                    "AllGather", A.bypass,
                    replica_groups=[list(range(N_CORES))],
                    ins=[vsh[:].opt()], outs=[vfull[:].opt()],
                )
            else:
                vfull = v_d

            ct = cpool.tile([P, 1, 3 * N_LEVELS_USED], f32)
            nc.sync.dma_start(out=ct[:], in_=c_d[:])

            m = CHUNK
            for it in range(PPP // m):
                xu = xpool.tile([P, m, 3], u16, tag="xu")
                nc.sync.dma_start(out=xu[:], in_=xr[:, it * m:(it + 1) * m, :])
                xt = xpool.tile([P, m, 3], f32, tag="xt")
                nc.scalar.copy(out=xt[:], in_=xu[:])  # exact ints < 2^16
                ot = opool.tile([P, m, 16], mybir.dt.int8 if OUT8 else bf16,
                                tag="ot")

                for l in range(N_LEVELS_USED):
                    c3 = ct[:, :, 3 * l:3 * l + 3]   # [S1, S2, 1]
                    t = wpool.tile([P, m, 3], f32, tag="t")
                    nc.vector.tensor_scalar_mul(out=t[:], in0=xt[:],
                                                scalar1=float(RES[l]) / XSCALE)
                    ti = wpool.tile([P, m, 3], i32, tag="ti")
                    nc.scalar.copy(out=ti[:], in_=t[:])    # round-to-nearest
                    bf = wpool.tile([P, m, 3], f32, tag="bf")
                    nc.scalar.copy(out=bf[:], in_=ti[:])
                    fx = wpool.tile([P, m, 3], f32, tag="fx")
                    nc.vector.tensor_tensor(out=fx[:], in0=bf[:], in1=t[:],
                                            op=A.is_gt)   # 1.0 where rounded up
                    nc.vector.tensor_tensor(out=bf[:], in0=bf[:], in1=fx[:],
                                            op=A.subtract)  # bf = floor(t)
                    nc.vector.tensor_tensor(out=t[:], in0=t[:], in1=bf[:],
                                            op=A.subtract)  # t = frac weights
                    nc.vector.tensor_tensor(out=fx[:], in0=bf[:],
                                            in1=c3.to_broadcast([P, m, 3]),
                                            op=A.mult)
                    vertf = wpool.tile([P, m, 1], f32, tag="vertf")
                    nc.vector.tensor_reduce(out=vertf[:], in_=fx[:],
                                            axis=mybir.AxisListType.X, op=A.add)

                    S1, S2 = VD[l] * VD[l], VD[l]
                    gf = gpool.tile([P, 4 * m, 4], f32, tag="gf")
                    if FULL_L[l]:
                        # one gather per point: rows [vert, vert+S1+S2+2)
                        # hold all 8 corners at static offsets i*S1+j*S2+k.
                        spr2 = (S1 + S2 + 2) * 2
                        vp = wpool.tile([P, m, 1], f32, tag="vp0")
                        nc.vector.tensor_scalar_add(out=vp[:], in0=vertf[:],
                                                    scalar1=float(VBASE[l]))
                        vi = wpool.tile([P, m, 1], i32, tag="vi0")
                        nc.scalar.copy(out=vi[:], in_=vp[:])
                        sm = min(FSM[l], m)
                        for sub in range(m // sm):
                            gr = spool.tile([P, FSNEL], bf16, tag="gsf")
                            gv = gr[:, :sm * spr2].rearrange(
                                "p (a r) -> p a r", r=spr2)
                            if NOGATHER:
                                nc.vector.memset(gv[:, :, 0:4], 0.0)
                            else:
                                for j0 in range(sm):
                                    nc.gpsimd.indirect_dma_start(
                                        out=gv[:, j0, :],
                                        out_offset=None,
                                        in_=vfull[:],
                                        in_offset=bass.IndirectOffsetOnAxis(
                                            ap=vi[:, sub * sm + j0, :],
                                            axis=0),
                                    )
                            for pair in range(4):
                                i, j = pair >> 1, pair & 1
                                a = pair * m + sub * sm
                                off = 2 * (i * S1 + j * S2)
                                nc.scalar.copy(
                                    out=gf[:, a:a + sm, :],
                                    in_=gv[:, :, off:off + 4])
                    elif SPAN:
                        # one gather per (point, i): the (S2+2)-row span
                        # starting at vert + i*S1 covers corners (i, j, k)
                        # for j,k in {0,1} at static row offsets j*S2 + k.
                        sp2 = (S2 + 2) * 2
                        vsi = []
                        for i in range(2):
                            vp = wpool.tile([P, m, 1], f32, tag=f"vp{i}")
                            nc.vector.tensor_scalar_add(
                                out=vp[:], in0=vertf[:],
                                scalar1=float(VBASE[l] + i * S1))
                            vi = wpool.tile([P, m, 1], i32, tag=f"vi{i}")
                            nc.scalar.copy(out=vi[:], in_=vp[:])
                            vsi.append(vi)
                        sm = min(SM, m)
                        for sub in range(m // sm):
                            gs = spool.tile([P, sm, 2, MAXSP * 2], bf16,
                                            tag="gs")
                            if NOGATHER:
                                nc.vector.memset(gs[:, :, :, :sp2], 0.0)
                            else:
                                for j0 in range(sm):
                                    for i in range(2):
                                        nc.gpsimd.indirect_dma_start(
                                            out=gs[:, j0, i, :sp2],
                                            out_offset=None,
                                            in_=vfull[:],
                                            in_offset=bass.IndirectOffsetOnAxis(
                                                ap=vsi[i][:, sub * sm + j0, :],
                                                axis=0),
                                        )
                            for pair in range(4):
                                i, j = pair >> 1, pair & 1
                                a = pair * m + sub * sm
                                nc.scalar.copy(
                                    out=gf[:, a:a + sm, :],
                                    in_=gs[:, :, i, 2 * j * S2:2 * j * S2 + 4])
                    else:
                        # 4 corner-pair index tiles: vert + base + i*S1 + j*S2
                        gp = gpool.tile([P, 4 * m, 4], bf16, tag="gp")
                        for pair in range(4):
                            i, j = pair >> 1, pair & 1
                            off = float(VBASE[l] + i * S1 + j * S2)
                            vp = wpool.tile([P, m, 1], f32, tag=f"vp{pair}")
                            nc.vector.tensor_scalar_add(out=vp[:], in0=vertf[:],
                                                        scalar1=off)
                            vi = wpool.tile([P, m, 1], i32, tag=f"vi{pair}")
                            nc.scalar.copy(out=vi[:], in_=vp[:])  # exact ints
                            if NOGATHER:
                                if pair == 0:
                                    nc.vector.memset(gp[:], 0.0)
                            else:
                                for j0 in range(m):
                                    nc.gpsimd.indirect_dma_start(
                                        out=gp[:, pair * m + j0, :],
                                        out_offset=None,
                                        in_=vfull[:],
                                        in_offset=bass.IndirectOffsetOnAxis(
                                            ap=vi[:, j0, :], axis=0),
                                    )
                        nc.vector.tensor_copy(out=gf[:], in_=gp[:])

                    # z-lerp within each pair: zt[p] = g0 + wz*(g1 - g0)
                    zd = gpool.tile([P, 4 * m, 2], f32, tag="zd")
                    nc.vector.tensor_tensor(out=zd[:], in0=gf[:, :, 2:4],
                                            in1=gf[:, :, 0:2], op=A.subtract)
                    zt = gpool.tile([P, 4 * m, 2], f32, tag="zt")
                    wz = t[:, :, 2:3]
                    for pr in range(4):
                        s = slice(pr * m, (pr + 1) * m)
                        nc.vector.tensor_tensor(
                            out=zd[:, s, :], in0=zd[:, s, :],
                            in1=wz.to_broadcast([P, m, 2]), op=A.mult)
                        nc.vector.tensor_tensor(
                            out=zt[:, s, :], in0=gf[:, pr * m:(pr + 1) * m, 0:2],
                            in1=zd[:, s, :], op=A.add)

                    # y-lerp: yt[i] = zt[i,0] + wy*(zt[i,1] - zt[i,0])
                    yt = gpool.tile([P, 2 * m, 2], f32, tag="yt")
                    wy = t[:, :, 1:2]
                    for i in range(2):
                        a = slice(2 * i * m, (2 * i + 1) * m)
                        b = slice((2 * i + 1) * m, (2 * i + 2) * m)
                        o = slice(i * m, (i + 1) * m)
                        nc.vector.tensor_tensor(out=yt[:, o, :], in0=zt[:, b, :],
                                                in1=zt[:, a, :], op=A.subtract)
                        nc.vector.tensor_tensor(
                            out=yt[:, o, :], in0=yt[:, o, :],
                            in1=wy.to_broadcast([P, m, 2]), op=A.mult)
                        nc.vector.tensor_tensor(out=yt[:, o, :], in0=yt[:, o, :],
                                                in1=zt[:, a, :], op=A.add)

                    # x-lerp: out_l = yt[0] + wx*(yt[1] - yt[0])
                    xd = wpool.tile([P, m, 2], f32, tag="xd")
                    wx = t[:, :, 0:1]
                    nc.vector.tensor_tensor(out=xd[:], in0=yt[:, m:2 * m, :],
                                            in1=yt[:, 0:m, :], op=A.subtract)
                    nc.vector.tensor_tensor(out=xd[:], in0=xd[:],
                                            in1=wx.to_broadcast([P, m, 2]),
                                            op=A.mult)
                    if OUT8:
                        xo = wpool.tile([P, m, 2], f32, tag="xo")
                        nc.vector.tensor_tensor(out=xo[:], in0=yt[:, 0:m, :],
                                                in1=xd[:], op=A.add)
                        nc.vector.tensor_scalar_mul(out=xo[:], in0=xo[:],
                                                    scalar1=OSCALE)
                        qi = wpool.tile([P, m, 2], i32, tag="qi")
                        nc.scalar.copy(out=qi[:], in_=xo[:])  # RNE, in [-127,127]
                        nc.scalar.copy(out=ot[:, :, 2 * l:2 * l + 2], in_=qi[:])
                    else:
                        nc.vector.tensor_tensor(out=ot[:, :, 2 * l:2 * l + 2],
                                                in0=yt[:, 0:m, :], in1=xd[:],
                                                op=A.add)

                nc.sync.dma_start(out=orr[:, it * m:(it + 1) * m, :],
                                  in_=ot[:])

    nc.compile()
    return nc


def _get_compiled():
    global _COMPILED
    if _COMPILED is None:
        _COMPILED = _compile()
    return _COMPILED


def kernel(x: np.ndarray, tables: np.ndarray, _want_trace: bool = False):
    nc = _get_compiled()
    x = np.asarray(x, dtype=np.float32)
    tables = np.asarray(tables, dtype=np.float32)
    # uint16 fixed-point encode: xq/65536 is within 1.6e-5 of x, and the
    # trilinear interpolant is continuous in x, so the output perturbation is
    # far below the bf16 table quantization already present.
    xq = np.ascontiguousarray((x * np.float32(XSCALE)).astype(np.uint16))
    vt = _build_packed_table(tables)
    consts = np.empty((P, 1, 3 * N_LEVELS_USED), np.float32)
    for l in range(N_LEVELS_USED):
        consts[:, :, 3 * l:3 * l + 3] = [float(VD[l] * VD[l]), float(VD[l]), 1.0]
    n_use = NPC * N_CORES
    in_maps = [{"x": xq[c * NPC:(c + 1) * NPC],
                "vtab": vt[c * VSHARD:(c + 1) * VSHARD] if AG else vt,
                "consts": consts}
               for c in range(N_CORES)]
    res = run_bass_kernel_spmd(nc, in_maps, list(range(N_CORES)),
                               trace=_want_trace)
    out = np.empty((n_use, 16), dtype=np.float32)
    for c in range(N_CORES):
        o = res.results[c]["out"].astype(np.float32)
        if OUT8:
            o *= np.float32(1.0 / OSCALE)
        out[c * NPC:(c + 1) * NPC] = o
    if _want_trace:
        return out, res
    return out
